# revision 20
# baseline (speedup 1.0000x reference)
"""CBOW negative-sampling loss kernel for Trainium2 (8 NeuronCores).

Strategy: data-parallel over batch (16384 -> 8 x 2048). Each batch row
needs 31 embedding rows (10 ctx + 1 center + 20 neg). The v1 baseline
gathered them with per-slot SWDGE indirect DMAs (one index per output
partition -> 496 Pool instructions/core at ~1.6 us each = ~774 us,
Pool-descriptor-generation bound). v2 replaces that with the custom
`dma_gather` SWDGE instruction (InstDMAGatherAnt, mlp library), which
gathers num_idxs rows in ONE Pool instruction (994 ns fixed + 0.34
ns/descriptor), so a whole 128-row tile (3968 rows) costs ~2.4 us of
Pool time and the critical path moves to the SDMA/HBM transfer.

dma_gather constraints and how we meet them:
  - int16 indices: vocab is 100k, but each half-core (8 tiles = 31744
    (row,slot) draws) can touch at most 31744 < 32768 unique rows, so
    the host dedups (table_id, vocab_row) keys per half and uploads a
    per-half fused sub-table (ctx_w/cen_w rows interleaved as
    first-seen) plus remapped int16 codes. Device-side traffic is
    unchanged (63488 row-reads/core); only the naming is compacted.
  - rows must be 256B-aligned: tables are cast to bf16 and padded to
    384 cols (768 B rows), pad cols zero-filled.
  - index layout: index i lives at wrapped[i%16, i//16], read from SBUF
    partitions 0-15 (tx Q7) and 16-31 (rx Q7) -> host replicates the
    [16, 248]-per-tile block across all 128 partitions.
  - destination: gathered row i -> out[i%128, i//128, :], so ordering
    i = slot*128 + p puts (batch row p, slot s) at partition p, free
    slot s -- exactly the per-partition layout the compute needs.

Compute per tile (all bf16 dense step-1 so DVE packs 2 elem/cycle):
  - ctx_sum: tree of 3 adds + 1 add (slots 0-4 + slots 5-9, then pair
    tree) instead of a strided 10-way reduce (strided kills packing).
  - prod = cn_embs * ctx_sum (broadcast over the 21 slots), bf16 out.
  - scores = reduce_sum(prod) over d -> [128, 21] f32.
  - ACT: exp(-0.1*pos) / exp(+0.1*neg) (sign+scale folded into the
    activation scale), then ln(1+e) with accum_out collapses the 21
    log-sigmoid terms into acc[:, t]. Exp/Ln share one function table.
Per-core output is [128, 16] per-row losses; the host means them.
"""

import sys

for _p in ("/opt/trn_rl_repo", "/root/.axon_site/_ro/trn_rl_repo"):
    if _p not in sys.path:
        sys.path.append(_p)

import numpy as np

VOCAB = 100000
D = 300
DPAD = 384  # bf16 row padded to 768 B (256B-aligned)
N_CTX = 10
N_NEG = 20
N_CN = 1 + N_NEG  # 21
N_SLOTS = N_CTX + N_CN  # 31
N_CORES = 8
BATCH = 16384
P = 128
B_CORE = BATCH // N_CORES  # 2048
N_TILES = B_CORE // P  # 16
IDX_PER_TILE = N_SLOTS * P  # 3968 gathers per tile
IDX_W = IDX_PER_TILE // 16  # 248 int16 per partition per tile
U_MAX = (N_TILES // 2) * IDX_PER_TILE  # 31744 rows per half sub-table
# slot groups per gather instruction (<=1024 idxs = ring capacity).
# Groups 0-1 fill the ctx tile (slots 0-9), groups 2-4 the cn tile
# (slots 10-30), so the ctx-sum tree only waits on its own two gathers.
SLOT_GROUPS = ((0, 5), (5, 10), (10, 18), (18, 26), (26, N_SLOTS))


def dma_gather_raw(
    nc, out_ap, in_ap, idxs_ap, num_idxs, elem_size, queue_num=0
):
    """bass.dma_gather minus the elem_size_bytes%256 assert: that check is a
    transpose-mode restriction misapplied to the non-transpose path (the
    firmware builds one descriptor of elem_size bytes per index; only the
    row STRIDE must be a multiple of 256B). Lets us gather 600B payloads
    from 768B-strided rows."""
    from concourse import mybir
    from concourse.bass import ap_utils

    g = nc.gpsimd
    g._assert_queue_num(queue_num)
    assert idxs_ap.dtype == mybir.dt.int16
    assert in_ap.dtype == out_ap.dtype
    assert ap_utils.ap_is_contiguous(in_ap.ap[1:])
    assert ap_utils.ap_is_contiguous(out_ap.ap[1:])
    assert ap_utils.ap_is_contiguous(idxs_ap.ap[1:])
    assert in_ap.ap[-1][1] == out_ap.ap[-1][1] == elem_size
    assert out_ap.ap[0][1] * out_ap.ap[1][1] == ((num_idxs + 127) // 128) * 128
    elem_step = in_ap.ap[0][0]
    stride_bytes = elem_step * mybir.dt.size(in_ap.dtype)
    stride_bytes_256 = stride_bytes // 256
    assert stride_bytes % 256 == 0 and 0 < stride_bytes_256 < 256
    _in_ap = g.lower_ap_dma(in_ap, for_custom_bir_dma=True)
    _idxs_ap = g.lower_ap(idxs_ap)
    _out_ap = g.lower_ap(out_ap)
    return g.add_instruction(
        mybir.InstDMAGatherAnt(
            name=nc.get_next_instruction_name(),
            ins=[*_in_ap, _idxs_ap, g.lower_val_access(g.to_reg(num_idxs))],
            outs=[_out_ap],
            transpose=False,
            num_idxs=num_idxs,
            elem_size=elem_size,
            stride_bytes_256=stride_bytes_256,
            gen_mode=0,
            single_packet=True,
            queue_num=queue_num,
            sbuf_tokens_per_rank=0,
            sbuf_free_dim_per_rank=0,
            sbuf_free_dim_pad_per_rank=0,
            sbuf_byte_offset=0,
        )
    )


def emit_cbow_body(nc, tc, idx16, sub0, sub1, signs, out, n_tiles):
    """Emit the per-core program body into an open TileContext.

    idx16: [P, n_tiles*IDX_W] int16 DRAM (remapped, wrapped, replicated)
    sub0:  [u_max, DPAD] bf16 DRAM -- fused sub-table for tiles 0..h-1
    sub1:  [u_max, DPAD] bf16 DRAM -- fused sub-table for tiles h..n-1
    signs: [P, N_CN] f32 DRAM -- [-0.1, +0.1 x20] replicated rows
    out:   [P, n_tiles] f32 DRAM -- out[p, t] = sum_i ln(1+exp(-x_i))
    """
    from concourse import mybir

    f32 = mybir.dt.float32
    bf16 = mybir.dt.bfloat16
    i16 = mybir.dt.int16
    add = mybir.AluOpType.add
    mult = mybir.AluOpType.mult
    half = n_tiles // 2
    with (
        tc.tile_pool(name="gctx", bufs=4) as gcpool,
        tc.tile_pool(name="gcn", bufs=4) as gnpool,
        tc.tile_pool(name="small", bufs=3) as spool,
        tc.tile_pool(name="accp", bufs=1) as apool,
    ):
        acc = apool.tile([P, n_tiles], f32)
        # raw scores for every tile, scaled by -+0.1 (x-folding): col
        # t*N_CN+0 = -0.1*pos_score, cols 1..20 = +0.1*neg_score.
        sc_all = apool.tile([P, n_tiles * N_CN], f32)
        idx_sb = apool.tile([P, n_tiles * IDX_W], i16)
        nc.sync.dma_start(out=idx_sb[:], in_=idx16[:])
        signs_sb = apool.tile([P, N_CN], f32)
        nc.sync.dma_start(out=signs_sb[:], in_=signs[:])
        n_gather = 0
        for t in range(n_tiles):
            sub = sub0 if t < half else sub1
            gc = gcpool.tile([P, N_CTX * D], bf16, tag="gc")
            gn = gnpool.tile([P, N_CN * D], bf16, tag="gn")
            # The runtime SWDGE descriptor ring holds only ~64 data
            # descriptors per SDMA lane, so one gather is capped at 1024
            # indices (65 descs/lane incl. the sem). Rotate the 4 SWDGE
            # queues so one queue's descriptor generation overlaps
            # another queue's DMA drain.
            col = t * IDX_W
            for s0, s1 in SLOT_GROUPS:
                w = (s1 - s0) * P // 16
                if s1 <= N_CTX:
                    out_ap = gc[:, s0 * D : s1 * D]
                else:
                    out_ap = gn[:, (s0 - N_CTX) * D : (s1 - N_CTX) * D]
                dma_gather_raw(
                    nc,
                    out_ap=out_ap.rearrange("p (s d) -> p s d", s=s1 - s0),
                    in_ap=sub[:, :D],
                    idxs_ap=idx_sb[:, col : col + w],
                    num_idxs=(s1 - s0) * P,
                    elem_size=D,
                    queue_num=n_gather % 4,
                )
                col += w
                n_gather += 1

            # ctx_sum[p, :] = sum of slots 0..9 (pairwise tree, dense bf16)
            c1 = spool.tile([P, 5 * D], bf16, tag="c1")
            nc.vector.tensor_tensor(
                out=c1[:], in0=gc[:, : 5 * D], in1=gc[:, 5 * D :], op=add
            )
            c2 = spool.tile([P, 2 * D], bf16, tag="c2")
            nc.vector.tensor_tensor(
                out=c2[:], in0=c1[:, : 2 * D], in1=c1[:, 2 * D : 4 * D], op=add
            )
            c3 = spool.tile([P, D], bf16, tag="c3")
            nc.vector.tensor_tensor(
                out=c3[:], in0=c2[:, :D], in1=c2[:, D : 2 * D], op=add
            )
            ctx = spool.tile([P, D], bf16, tag="ctx")
            nc.vector.tensor_tensor(
                out=ctx[:], in0=c3[:], in1=c1[:, 4 * D : 5 * D], op=add
            )

            # prod[p, n, d] = cn[p, n, d] * ctx[p, d]; then a pairwise-add
            # tree halves d before the (1x-only) reduce: tensor_reduce has
            # no packed uop, so shrink its input with 2x tensor_tensor
            # adds first. (The +-0.1 sign/scale is applied to the [P, 336]
            # scores at the end -- a tensor_scalar here would auto-select
            # the 2-port 4x mode and stall multi-us against the SWDGE
            # descriptor-ring SBUF traffic of the concurrent gathers.)
            prod = spool.tile([P, N_CN * D], bf16, tag="prod")
            nc.vector.tensor_tensor(
                out=prod.rearrange("p (n d) -> p n d", n=N_CN),
                in0=gn.rearrange("p (n d) -> p n d", n=N_CN),
                in1=ctx.unsqueeze(1).broadcast_to([P, N_CN, D]),
                op=mult,
            )
            h1 = spool.tile([P, N_CN * 150], bf16, tag="h1")
            p3 = prod.rearrange("p (n d) -> p n d", n=N_CN)
            nc.vector.tensor_tensor(
                out=h1.rearrange("p (n d) -> p n d", n=N_CN),
                in0=p3[:, :, 0:150],
                in1=p3[:, :, 150:300],
                op=add,
            )
            h2 = spool.tile([P, N_CN * 75], bf16, tag="h2")
            h13 = h1.rearrange("p (n d) -> p n d", n=N_CN)
            nc.vector.tensor_tensor(
                out=h2.rearrange("p (n d) -> p n d", n=N_CN),
                in0=h13[:, :, 0:75],
                in1=h13[:, :, 75:150],
                op=add,
            )
            nc.vector.tensor_reduce(
                out=sc_all[:, t * N_CN : (t + 1) * N_CN],
                in_=h2.rearrange("p (n d) -> p n d", n=N_CN),
                axis=mybir.AxisListType.X,
                op=add,
            )
            if t == half - 1 or t == n_tiles - 1:
                # Softplus for the finished half, batched so ACT loads
                # each function table once per half and the first half's
                # tail overlaps the second half's gathers/DVE:
                # acc[:, t] = sum_n ln(1 + exp(+-0.1 * sc_all[:, t, n])).
                t0 = 0 if t == half - 1 else half
                hw = half * N_CN
                nc.vector.tensor_tensor(
                    out=sc_all[:, t0 * N_CN : t0 * N_CN + hw].rearrange(
                        "p (t n) -> p t n", t=half
                    ),
                    in0=sc_all[:, t0 * N_CN : t0 * N_CN + hw].rearrange(
                        "p (t n) -> p t n", t=half
                    ),
                    in1=signs_sb.unsqueeze(1).broadcast_to([P, half, N_CN]),
                    op=mult,
                )
                ex_all = spool.tile([P, hw], f32, tag="ex")
                nc.scalar.activation(
                    out=ex_all[:],
                    in_=sc_all[:, t0 * N_CN : t0 * N_CN + hw],
                    func=mybir.ActivationFunctionType.Exp,
                )
                for tt in range(t0, t0 + half):
                    lns = spool.tile([P, N_CN], f32, tag="lns")
                    nc.scalar.activation(
                        out=lns[:],
                        in_=ex_all[:, (tt - t0) * N_CN : (tt - t0 + 1) * N_CN],
                        func=mybir.ActivationFunctionType.Ln,
                        bias=1.0,
                        accum_out=acc[:, tt : tt + 1],
                    )
        nc.sync.dma_start(out=out[:], in_=acc[:])


def build_program(n_tiles=N_TILES, u_max=U_MAX, n_cores=N_CORES):
    from concourse import mybir
    import concourse.bacc as bacc
    import concourse.tile as tile

    nc = bacc.Bacc(
        "TRN2",
        target_bir_lowering=False,
        debug=False,
        enable_asserts=False,
        num_devices=n_cores,
        num_swdge_queues=4,
    )
    idx16 = nc.dram_tensor(
        "idx16", [P, n_tiles * IDX_W], mybir.dt.int16, kind="ExternalInput"
    ).ap()
    sub0 = nc.dram_tensor(
        "sub0", [u_max, DPAD], mybir.dt.bfloat16, kind="ExternalInput"
    ).ap()
    sub1 = nc.dram_tensor(
        "sub1", [u_max, DPAD], mybir.dt.bfloat16, kind="ExternalInput"
    ).ap()
    signs = nc.dram_tensor(
        "signs", [P, N_CN], mybir.dt.float32, kind="ExternalInput"
    ).ap()
    out = nc.dram_tensor(
        "out", [P, n_tiles], mybir.dt.float32, kind="ExternalOutput"
    ).ap()
    with tile.TileContext(nc) as tc:
        emit_cbow_body(nc, tc, idx16, sub0, sub1, signs, out, n_tiles)
    nc.compile()
    return nc


_NC_CACHE = {}


def _get_program():
    if "nc" not in _NC_CACHE:
        _NC_CACHE["nc"] = build_program()
    return _NC_CACHE["nc"]


def pack_keys(context, center, negatives):
    """[BATCH, N_SLOTS] int32 fused keys: ctx rows (cols 0..9) keep their
    vocab id; center/neg rows (cols 10..30) get +VOCAB (cen_w table)."""
    ctx = np.asarray(context, dtype=np.int32).reshape(BATCH, N_CTX)
    cen = np.asarray(center, dtype=np.int32).reshape(BATCH, 1) + VOCAB
    neg = np.asarray(negatives, dtype=np.int32).reshape(BATCH, N_NEG) + VOCAB
    return np.ascontiguousarray(np.concatenate([ctx, cen, neg], axis=1))


def build_wall(context_weight, center_weight):
    """[2*VOCAB, DPAD] bf16 zero-padded fused weight table."""
    import ml_dtypes

    wall = np.zeros((2 * VOCAB, DPAD), dtype=ml_dtypes.bfloat16)
    wall[:VOCAB, :D] = np.asarray(context_weight, dtype=np.float32)
    wall[VOCAB:, :D] = np.asarray(center_weight, dtype=np.float32)
    return wall


def prepare_core_inputs(keys_core, wall, n_tiles=N_TILES, u_max=U_MAX):
    """Build one core's device inputs.

    keys_core: [n_tiles*P, N_SLOTS] int32 fused keys (batch-tile order:
        row t*P+p -> tile t, partition p).
    wall: [2*VOCAB, DPAD] bf16 padded fused table.
    Returns dict(idx16=[P, n_tiles*IDX_W] i16, sub0, sub1=[u_max, DPAD] bf16).
    """
    import ml_dtypes

    half = n_tiles // 2
    subs = []
    wrapped_cols = []
    for h in (0, 1):
        blk = keys_core[h * half * P : (h + 1) * half * P]  # [half*P, N_SLOTS]
        uniq, inv = np.unique(blk, return_inverse=True)
        assert len(uniq) <= u_max
        sub = np.zeros((u_max, DPAD), dtype=ml_dtypes.bfloat16)
        sub[: len(uniq)] = wall[uniq]
        subs.append(sub)
        inv16 = inv.astype(np.int16).reshape(half, P, N_SLOTS)
        for tt in range(half):
            for s0, s1 in SLOT_GROUPS:
                flat = inv16[tt].T[s0:s1].ravel()  # i = (s-s0)*P + p
                wrapped_cols.append(flat.reshape(len(flat) // 16, 16).T)
    idx16 = np.tile(np.concatenate(wrapped_cols, axis=1), (P // 16, 1))
    return {
        "idx16": np.ascontiguousarray(idx16),
        "sub0": subs[0],
        "sub1": subs[1],
        "signs": np.tile(
            np.array([[-0.1] + [0.1] * N_NEG], dtype=np.float32), (P, 1)
        ),
    }


def make_in_maps(context, center, negatives, context_weight, center_weight):
    keys = pack_keys(context, center, negatives).reshape(
        N_CORES, B_CORE, N_SLOTS
    )
    wall = build_wall(context_weight, center_weight)
    return [prepare_core_inputs(keys[c], wall) for c in range(N_CORES)]


def kernel(context, center, negatives, context_weight, center_weight):
    from concourse import bass_utils

    nc = _get_program()
    in_maps = make_in_maps(
        context, center, negatives, context_weight, center_weight
    )
    res = bass_utils.run_bass_kernel_spmd(nc, in_maps, core_ids=list(range(N_CORES)))
    acc = np.stack([r["out"] for r in res.results])  # [N_CORES, P, N_TILES]
    return np.array(acc.sum(dtype=np.float64) / BATCH, dtype=np.float32)


# revision 21
# speedup vs baseline: 1.1303x; 1.1303x over previous
"""CBOW negative-sampling loss kernel for Trainium2 (8 NeuronCores).

Strategy: data-parallel over batch (16384 -> 8 x 2048). Each batch row
needs 31 embedding rows (10 ctx + 1 center + 20 neg). The v1 baseline
gathered them with per-slot SWDGE indirect DMAs (one index per output
partition -> 496 Pool instructions/core at ~1.6 us each = ~774 us,
Pool-descriptor-generation bound). v2 replaces that with the custom
`dma_gather` SWDGE instruction (InstDMAGatherAnt, mlp library), which
gathers num_idxs rows in ONE Pool instruction (994 ns fixed + 0.34
ns/descriptor), so a whole 128-row tile (3968 rows) costs ~2.4 us of
Pool time and the critical path moves to the SDMA/HBM transfer.

dma_gather constraints and how we meet them:
  - int16 indices: vocab is 100k, but each half-core (8 tiles = 31744
    (row,slot) draws) can touch at most 31744 < 32768 unique rows, so
    the host dedups (table_id, vocab_row) keys per half and uploads a
    per-half fused sub-table (ctx_w/cen_w rows interleaved as
    first-seen) plus remapped int16 codes. Device-side traffic is
    unchanged (63488 row-reads/core); only the naming is compacted.
  - rows must be 256B-aligned: tables are cast to bf16 and padded to
    384 cols (768 B rows), pad cols zero-filled.
  - index layout: index i lives at wrapped[i%16, i//16], read from SBUF
    partitions 0-15 (tx Q7) and 16-31 (rx Q7) -> host replicates the
    [16, 248]-per-tile block across all 128 partitions.
  - destination: gathered row i -> out[i%128, i//128, :], so ordering
    i = slot*128 + p puts (batch row p, slot s) at partition p, free
    slot s -- exactly the per-partition layout the compute needs.

Compute per tile (all bf16 dense step-1 so DVE packs 2 elem/cycle):
  - ctx_sum: tree of 3 adds + 1 add (slots 0-4 + slots 5-9, then pair
    tree) instead of a strided 10-way reduce (strided kills packing).
  - prod = cn_embs * ctx_sum (broadcast over the 21 slots), bf16 out.
  - scores = reduce_sum(prod) over d -> [128, 21] f32.
  - ACT: exp(-0.1*pos) / exp(+0.1*neg) (sign+scale folded into the
    activation scale), then ln(1+e) with accum_out collapses the 21
    log-sigmoid terms into acc[:, t]. Exp/Ln share one function table.
Per-core output is [128, 16] per-row losses; the host means them.
"""

import sys

for _p in ("/opt/trn_rl_repo", "/root/.axon_site/_ro/trn_rl_repo"):
    if _p not in sys.path:
        sys.path.append(_p)

import numpy as np

VOCAB = 100000
D = 300
DPAD = 384  # bf16 row padded to 768 B (256B-aligned)
N_CTX = 10
N_NEG = 20
N_CN = 1 + N_NEG  # 21
N_SLOTS = N_CTX + N_CN  # 31
N_CORES = 8
BATCH = 16384
P = 128
B_CORE = BATCH // N_CORES  # 2048
N_TILES = B_CORE // P  # 16
IDX_PER_TILE = N_SLOTS * P  # 3968 gathers per tile
IDX_W = IDX_PER_TILE // 16  # 248 int16 per partition per tile
U_MAX = (N_TILES // 2) * IDX_PER_TILE  # 31744 rows per half sub-table
# slot groups per gather instruction (<=1024 idxs = ring capacity).
# Groups 0-1 fill the ctx tile (slots 0-9), groups 2-4 the cn tile
# (slots 10-30), so the ctx-sum tree only waits on its own two gathers.
SLOT_GROUPS = ((0, 5), (5, 10), (10, 18), (18, 26), (26, N_SLOTS))


def dma_gather_raw(
    nc, out_ap, in_ap, idxs_ap, num_idxs, elem_size, queue_num=0
):
    """bass.dma_gather minus the elem_size_bytes%256 assert: that check is a
    transpose-mode restriction misapplied to the non-transpose path (the
    firmware builds one descriptor of elem_size bytes per index; only the
    row STRIDE must be a multiple of 256B). Lets us gather 600B payloads
    from 768B-strided rows."""
    from concourse import mybir
    from concourse.bass import ap_utils

    g = nc.gpsimd
    g._assert_queue_num(queue_num)
    assert idxs_ap.dtype == mybir.dt.int16
    assert in_ap.dtype == out_ap.dtype
    assert ap_utils.ap_is_contiguous(in_ap.ap[1:])
    assert ap_utils.ap_is_contiguous(out_ap.ap[1:])
    assert ap_utils.ap_is_contiguous(idxs_ap.ap[1:])
    assert in_ap.ap[-1][1] == out_ap.ap[-1][1] == elem_size
    assert out_ap.ap[0][1] * out_ap.ap[1][1] == ((num_idxs + 127) // 128) * 128
    elem_step = in_ap.ap[0][0]
    stride_bytes = elem_step * mybir.dt.size(in_ap.dtype)
    stride_bytes_256 = stride_bytes // 256
    assert stride_bytes % 256 == 0 and 0 < stride_bytes_256 < 256
    _in_ap = g.lower_ap_dma(in_ap, for_custom_bir_dma=True)
    _idxs_ap = g.lower_ap(idxs_ap)
    _out_ap = g.lower_ap(out_ap)
    return g.add_instruction(
        mybir.InstDMAGatherAnt(
            name=nc.get_next_instruction_name(),
            ins=[*_in_ap, _idxs_ap, g.lower_val_access(g.to_reg(num_idxs))],
            outs=[_out_ap],
            transpose=False,
            num_idxs=num_idxs,
            elem_size=elem_size,
            stride_bytes_256=stride_bytes_256,
            gen_mode=0,
            single_packet=True,
            queue_num=queue_num,
            sbuf_tokens_per_rank=0,
            sbuf_free_dim_per_rank=0,
            sbuf_free_dim_pad_per_rank=0,
            sbuf_byte_offset=0,
        )
    )


def emit_cbow_body(nc, tc, idx16, sub0, sub1, signs, out, n_tiles):
    """Emit the per-core program body into an open TileContext.

    idx16: [P, n_tiles*IDX_W] int16 DRAM (remapped, wrapped, replicated)
    sub0:  [u_max, DPAD] bf16 DRAM -- fused sub-table for tiles 0..h-1
    sub1:  [u_max, DPAD] bf16 DRAM -- fused sub-table for tiles h..n-1
    signs: [P, N_CN] f32 DRAM -- [-0.1, +0.1 x20] replicated rows
    out:   [P, n_tiles] f32 DRAM -- out[p, t] = sum_i ln(1+exp(-x_i))
    """
    from concourse import mybir

    f32 = mybir.dt.float32
    bf16 = mybir.dt.bfloat16
    i16 = mybir.dt.int16
    add = mybir.AluOpType.add
    mult = mybir.AluOpType.mult
    half = n_tiles // 2
    with (
        tc.tile_pool(name="gctx", bufs=4) as gcpool,
        tc.tile_pool(name="gcn", bufs=4) as gnpool,
        tc.tile_pool(name="small", bufs=3) as spool,
        tc.tile_pool(name="accp", bufs=1) as apool,
    ):
        acc = apool.tile([P, n_tiles], f32)
        # raw scores for every tile, scaled by -+0.1 (x-folding): col
        # t*N_CN+0 = -0.1*pos_score, cols 1..20 = +0.1*neg_score.
        sc_all = apool.tile([P, n_tiles * N_CN], f32)
        idx_sb = apool.tile([P, n_tiles * IDX_W], i16)
        nc.sync.dma_start(out=idx_sb[:], in_=idx16[:])
        signs_sb = apool.tile([P, N_CN], f32)
        nc.sync.dma_start(out=signs_sb[:], in_=signs[:])
        n_gather = 0
        for t in range(n_tiles):
            sub = sub0 if t < half else sub1
            gc = gcpool.tile([P, N_CTX * D], bf16, tag="gc")
            gn = gnpool.tile([P, N_CN * D], bf16, tag="gn")
            # The runtime SWDGE descriptor ring holds only ~64 data
            # descriptors per SDMA lane, so one gather is capped at 1024
            # indices (65 descs/lane incl. the sem). Rotate the 4 SWDGE
            # queues so one queue's descriptor generation overlaps
            # another queue's DMA drain.
            col = t * IDX_W
            for s0, s1 in SLOT_GROUPS:
                w = (s1 - s0) * P // 16
                if s1 <= N_CTX:
                    out_ap = gc[:, s0 * D : s1 * D]
                else:
                    out_ap = gn[:, (s0 - N_CTX) * D : (s1 - N_CTX) * D]
                dma_gather_raw(
                    nc,
                    out_ap=out_ap.rearrange("p (s d) -> p s d", s=s1 - s0),
                    in_ap=sub[:, :D],
                    idxs_ap=idx_sb[:, col : col + w],
                    num_idxs=(s1 - s0) * P,
                    elem_size=D,
                    queue_num=n_gather % 4,
                )
                col += w
                n_gather += 1

            # ctx_sum[p, :] = sum of slots 0..9 (pairwise tree, dense bf16)
            c1 = spool.tile([P, 5 * D], bf16, tag="c1")
            nc.vector.tensor_tensor(
                out=c1[:], in0=gc[:, : 5 * D], in1=gc[:, 5 * D :], op=add
            )
            c2 = spool.tile([P, 2 * D], bf16, tag="c2")
            nc.vector.tensor_tensor(
                out=c2[:], in0=c1[:, : 2 * D], in1=c1[:, 2 * D : 4 * D], op=add
            )
            c3 = spool.tile([P, D], bf16, tag="c3")
            nc.vector.tensor_tensor(
                out=c3[:], in0=c2[:, :D], in1=c2[:, D : 2 * D], op=add
            )
            ctx = spool.tile([P, D], bf16, tag="ctx")
            nc.vector.tensor_tensor(
                out=ctx[:], in0=c3[:], in1=c1[:, 4 * D : 5 * D], op=add
            )

            # prod[p, n, d] = cn[p, n, d] * ctx[p, d]; then a pairwise-add
            # tree halves d before the (1x-only) reduce: tensor_reduce has
            # no packed uop, so shrink its input with 2x tensor_tensor
            # adds first. (The +-0.1 sign/scale is applied to the [P, 336]
            # scores at the end -- a tensor_scalar here would auto-select
            # the 2-port 4x mode and stall multi-us against the SWDGE
            # descriptor-ring SBUF traffic of the concurrent gathers.)
            prod = spool.tile([P, N_CN * D], bf16, tag="prod")
            nc.vector.tensor_tensor(
                out=prod.rearrange("p (n d) -> p n d", n=N_CN),
                in0=gn.rearrange("p (n d) -> p n d", n=N_CN),
                in1=ctx.unsqueeze(1).broadcast_to([P, N_CN, D]),
                op=mult,
            )
            h1 = spool.tile([P, N_CN * 150], bf16, tag="h1")
            p3 = prod.rearrange("p (n d) -> p n d", n=N_CN)
            nc.vector.tensor_tensor(
                out=h1.rearrange("p (n d) -> p n d", n=N_CN),
                in0=p3[:, :, 0:150],
                in1=p3[:, :, 150:300],
                op=add,
            )
            h2 = spool.tile([P, N_CN * 75], bf16, tag="h2")
            h13 = h1.rearrange("p (n d) -> p n d", n=N_CN)
            nc.vector.tensor_tensor(
                out=h2.rearrange("p (n d) -> p n d", n=N_CN),
                in0=h13[:, :, 0:75],
                in1=h13[:, :, 75:150],
                op=add,
            )
            nc.vector.tensor_reduce(
                out=sc_all[:, t * N_CN : (t + 1) * N_CN],
                in_=h2.rearrange("p (n d) -> p n d", n=N_CN),
                axis=mybir.AxisListType.X,
                op=add,
            )
        # Apply the +-0.1 sign/scale to all raw scores at once (the accum
        # target is -x_n: center gets -0.1, negatives +0.1), then the
        # softplus tail, batched so ACT loads each function table once:
        # acc[:, t] = sum_n ln(1 + exp(sc_all[:, t, n])).
        nc.vector.tensor_tensor(
            out=sc_all.rearrange("p (t n) -> p t n", t=n_tiles),
            in0=sc_all.rearrange("p (t n) -> p t n", t=n_tiles),
            in1=signs_sb.unsqueeze(1).broadcast_to([P, n_tiles, N_CN]),
            op=mult,
        )
        ex_all = apool.tile([P, n_tiles * N_CN], f32)
        nc.scalar.activation(
            out=ex_all[:],
            in_=sc_all[:],
            func=mybir.ActivationFunctionType.Exp,
        )
        for t in range(n_tiles):
            lns = spool.tile([P, N_CN], f32, tag="lns")
            nc.scalar.activation(
                out=lns[:],
                in_=ex_all[:, t * N_CN : (t + 1) * N_CN],
                func=mybir.ActivationFunctionType.Ln,
                bias=1.0,
                accum_out=acc[:, t : t + 1],
            )
        nc.sync.dma_start(out=out[:], in_=acc[:])


def build_program(n_tiles=N_TILES, u_max=U_MAX, n_cores=N_CORES):
    from concourse import mybir
    import concourse.bacc as bacc
    import concourse.tile as tile

    nc = bacc.Bacc(
        "TRN2",
        target_bir_lowering=False,
        debug=False,
        enable_asserts=False,
        num_devices=n_cores,
        num_swdge_queues=4,
    )
    idx16 = nc.dram_tensor(
        "idx16", [P, n_tiles * IDX_W], mybir.dt.int16, kind="ExternalInput"
    ).ap()
    sub0 = nc.dram_tensor(
        "sub0", [u_max, DPAD], mybir.dt.bfloat16, kind="ExternalInput"
    ).ap()
    sub1 = nc.dram_tensor(
        "sub1", [u_max, DPAD], mybir.dt.bfloat16, kind="ExternalInput"
    ).ap()
    signs = nc.dram_tensor(
        "signs", [P, N_CN], mybir.dt.float32, kind="ExternalInput"
    ).ap()
    out = nc.dram_tensor(
        "out", [P, n_tiles], mybir.dt.float32, kind="ExternalOutput"
    ).ap()
    with tile.TileContext(nc) as tc:
        emit_cbow_body(nc, tc, idx16, sub0, sub1, signs, out, n_tiles)
    nc.compile()
    return nc


_NC_CACHE = {}


def _get_program():
    if "nc" not in _NC_CACHE:
        _NC_CACHE["nc"] = build_program()
    return _NC_CACHE["nc"]


def pack_keys(context, center, negatives):
    """[BATCH, N_SLOTS] int32 fused keys: ctx rows (cols 0..9) keep their
    vocab id; center/neg rows (cols 10..30) get +VOCAB (cen_w table)."""
    ctx = np.asarray(context, dtype=np.int32).reshape(BATCH, N_CTX)
    cen = np.asarray(center, dtype=np.int32).reshape(BATCH, 1) + VOCAB
    neg = np.asarray(negatives, dtype=np.int32).reshape(BATCH, N_NEG) + VOCAB
    return np.ascontiguousarray(np.concatenate([ctx, cen, neg], axis=1))


def build_wall(context_weight, center_weight):
    """[2*VOCAB, DPAD] bf16 zero-padded fused weight table."""
    import ml_dtypes

    wall = np.zeros((2 * VOCAB, DPAD), dtype=ml_dtypes.bfloat16)
    wall[:VOCAB, :D] = np.asarray(context_weight, dtype=np.float32)
    wall[VOCAB:, :D] = np.asarray(center_weight, dtype=np.float32)
    return wall


def prepare_core_inputs(keys_core, wall, n_tiles=N_TILES, u_max=U_MAX):
    """Build one core's device inputs.

    keys_core: [n_tiles*P, N_SLOTS] int32 fused keys (batch-tile order:
        row t*P+p -> tile t, partition p).
    wall: [2*VOCAB, DPAD] bf16 padded fused table.
    Returns dict(idx16=[P, n_tiles*IDX_W] i16, sub0, sub1=[u_max, DPAD] bf16).
    """
    import ml_dtypes

    half = n_tiles // 2
    subs = []
    wrapped_cols = []
    for h in (0, 1):
        blk = keys_core[h * half * P : (h + 1) * half * P]  # [half*P, N_SLOTS]
        uniq, inv = np.unique(blk, return_inverse=True)
        assert len(uniq) <= u_max
        sub = np.zeros((u_max, DPAD), dtype=ml_dtypes.bfloat16)
        sub[: len(uniq)] = wall[uniq]
        subs.append(sub)
        inv16 = inv.astype(np.int16).reshape(half, P, N_SLOTS)
        for tt in range(half):
            for s0, s1 in SLOT_GROUPS:
                flat = inv16[tt].T[s0:s1].ravel()  # i = (s-s0)*P + p
                wrapped_cols.append(flat.reshape(len(flat) // 16, 16).T)
    idx16 = np.tile(np.concatenate(wrapped_cols, axis=1), (P // 16, 1))
    return {
        "idx16": np.ascontiguousarray(idx16),
        "sub0": subs[0],
        "sub1": subs[1],
        "signs": np.tile(
            np.array([[-0.1] + [0.1] * N_NEG], dtype=np.float32), (P, 1)
        ),
    }


def make_in_maps(context, center, negatives, context_weight, center_weight):
    keys = pack_keys(context, center, negatives).reshape(
        N_CORES, B_CORE, N_SLOTS
    )
    wall = build_wall(context_weight, center_weight)
    return [prepare_core_inputs(keys[c], wall) for c in range(N_CORES)]


def kernel(context, center, negatives, context_weight, center_weight):
    from concourse import bass_utils

    nc = _get_program()
    in_maps = make_in_maps(
        context, center, negatives, context_weight, center_weight
    )
    res = bass_utils.run_bass_kernel_spmd(nc, in_maps, core_ids=list(range(N_CORES)))
    acc = np.stack([r["out"] for r in res.results])  # [N_CORES, P, N_TILES]
    return np.array(acc.sum(dtype=np.float64) / BATCH, dtype=np.float32)
